# revision 27
# baseline (speedup 1.0000x reference)
"""DCRNN baseline (GraphGRU over road graph) as a Bass/Tile kernel on 8 TRN2 cores.

Model (per reference):
    per step l:  Ah = A @ h            [B,N,H]   (graph aggregation)
                 gates = xt@W_x + Ah@W_h + biases
                 z = sig, r = sig, n = tanh(nx + r*nh)
                 h = (1-z)*n + z*h
    head:        y = (h @ W_head + b_head).transpose(0,2,1)   [B,HOR,N]

Sharding: data-parallel over batch B=32 across 8 cores (B_loc=4). A and all
weights replicated; no collectives.

Per-core layouts (free dim "(b,i)" is b-major, 4*1024 = 4096 cols):
    Hmat  [j, (b,h)]  : 8 j-tiles [128, 512]  — lhsT for the A@h matmul
    A_T   [j, i]      : 8 j-tiles [128, 1024] — rhs   for the A@h matmul
    AhT   [h, (b,i)]  : [128, 4096]           — MM1 out, rhs for gate matmul
    gates [g, (b,i)]  : PSUM tiles per 512-chunk; z|r share one psum tile
    Hrow  [h, (b,i)]  : [128, 4096]           — hidden state for elementwise
    per-step transpose Hrow -> Hmat via xbar DMA transpose (bf16)

Matmul path runs bf16 (fp32 PSUM accumulation); z/r/nx biases ride a ones-row
in the x-projection matmul; the n-gate h-bias is applied in the fused
scalar_tensor_tensor op.
"""

import numpy as np

B, L, N, F, H, HOR = 32, 12, 1024, 2, 128, 12
NCORES = 8
BLOC = B // NCORES          # 4 batch elements per core
NB = BLOC * N               # 4096 free columns "(b,i)"
JT = N // 128               # 8 j-tiles
CHUNK = 512                 # free-dim chunk for gate PSUM waves
NCH = NB // CHUNK           # 8 chunks
G3 = 3 * H

_COMPILED = None


def _build_program():
    import concourse.bass as bass
    import concourse.mybir as mybir
    import concourse.tile as tile
    from concourse import bacc

    f32 = mybir.dt.float32
    bf16 = mybir.dt.bfloat16
    AF = mybir.ActivationFunctionType
    ALU = mybir.AluOpType

    nc = bacc.Bacc("TRN2", target_bir_lowering=False, debug=False,
                   num_devices=NCORES)

    # xt rows: [x_f0; x_f1; ones]
    xt = nc.dram_tensor("xt", [L, F + 1, NB], bf16, kind="ExternalInput").ap()
    a_t = nc.dram_tensor("a_t", [N, N], bf16, kind="ExternalInput").ap()
    w_h = nc.dram_tensor("w_h", [H, G3], bf16, kind="ExternalInput").ap()
    # w_x rows: [W_x0; W_x1; bias_row] with bias_row = [bz | br | bxn]
    w_x = nc.dram_tensor("w_x", [F + 1, G3], bf16, kind="ExternalInput").ap()
    bhn_d = nc.dram_tensor("bhn", [H, 1], f32, kind="ExternalInput").ap()
    w_head = nc.dram_tensor("w_head", [H, HOR], bf16, kind="ExternalInput").ap()
    b_head = nc.dram_tensor("b_head", [HOR, 1], f32, kind="ExternalInput").ap()
    y = nc.dram_tensor("y", [BLOC, HOR, N], f32, kind="ExternalOutput").ap()

    with tile.TileContext(nc) as tc:
        with (
            tc.tile_pool(name="singles", bufs=1) as singles,
            tc.tile_pool(name="state", bufs=2) as state,
            tc.tile_pool(name="xtp", bufs=3) as xtp,
            tc.tile_pool(name="yout", bufs=2) as yout,
            tc.tile_pool(name="elw", bufs=3) as elw,
            tc.tile_pool(name="ps_ah", bufs=2, space="PSUM") as ps_ah,
            tc.tile_pool(name="ps_zr", bufs=1, space="PSUM") as ps_zr,
            tc.tile_pool(name="ps_nh", bufs=2, space="PSUM") as ps_nh,
            tc.tile_pool(name="ps_nx", bufs=2, space="PSUM") as ps_nx,
        ):
            # ---- resident constants ----
            at_sb = singles.tile([128, JT, N], bf16)
            nc.gpsimd.dma_start(
                out=at_sb, in_=a_t.rearrange("(t p) i -> p t i", p=128))
            wh_sb = singles.tile([H, G3], bf16)
            nc.gpsimd.dma_start(out=wh_sb, in_=w_h)
            wx_sb = singles.tile([F + 1, G3], bf16)
            nc.gpsimd.dma_start(out=wx_sb, in_=w_x)
            bhn_sb = singles.tile([H, 1], f32)
            nc.sync.dma_start(out=bhn_sb, in_=bhn_d)
            whead_sb = singles.tile([H, HOR], bf16)
            nc.sync.dma_start(out=whead_sb, in_=w_head)
            bhead_sb = singles.tile([HOR, 1], f32)
            nc.sync.dma_start(out=bhead_sb, in_=b_head)

            # ---- recurrence (step 0 special-cased: h == 0) ----
            hmat = None
            hrow = None

            for l in range(L):
                first = l == 0
                aht = state.tile([128, NB], bf16, tag="aht")
                hmat_new = state.tile([128, JT, 512], bf16, tag="hmat")
                hrow_new = state.tile([128, NB], bf16, tag="hrow")

                for c in range(NCH):
                    b, half = divmod(c, N // CHUNK)
                    cs = slice(c * CHUNK, (c + 1) * CHUNK)
                    is_ = slice(half * CHUNK, (half + 1) * CHUNK)

                    if not first:
                        pah = ps_ah.tile([128, CHUNK], f32, tag="pah")
                        for jt in range(JT):
                            nc.tensor.matmul(
                                pah,
                                lhsT=hmat[:, jt, b * H:(b + 1) * H],
                                rhs=at_sb[:, jt, is_],
                                start=(jt == 0), stop=(jt == JT - 1))
                        if c % 2 == 0:
                            nc.scalar.copy(aht[:, cs], pah)
                        else:
                            nc.vector.tensor_copy(aht[:, cs], pah)

                    xt_c = xtp.tile([F + 1, CHUNK], bf16, tag="xt")
                    nc.sync.dma_start(out=xt_c, in_=xt[l][:, cs])

                    # gate pre-activations in PSUM; z|r share one tile,
                    # biases ride the ones-row of the x projection
                    rhs_ah = aht[:, cs]
                    pzr = ps_zr.tile([128, 2 * CHUNK], f32, tag="pzr")
                    pnx = ps_nx.tile([128, CHUNK], f32, tag="pnx")
                    nc.tensor.matmul(pzr[:, 0:CHUNK], lhsT=wx_sb[:, 0:H],
                                     rhs=xt_c, start=True, stop=first)
                    nc.tensor.matmul(pzr[:, CHUNK:2 * CHUNK],
                                     lhsT=wx_sb[:, H:2 * H],
                                     rhs=xt_c, start=True, stop=first)
                    nc.tensor.matmul(pnx, lhsT=wx_sb[:, 2 * H:G3],
                                     rhs=xt_c, start=True, stop=True)
                    if not first:
                        pnh = ps_nh.tile([128, CHUNK], f32, tag="pnh")
                        nc.tensor.matmul(pzr[:, 0:CHUNK], lhsT=wh_sb[:, 0:H],
                                         rhs=rhs_ah, start=False, stop=True)
                        nc.tensor.matmul(pzr[:, CHUNK:2 * CHUNK],
                                         lhsT=wh_sb[:, H:2 * H],
                                         rhs=rhs_ah, start=False, stop=True)
                        nc.tensor.matmul(pnh, lhsT=wh_sb[:, 2 * H:G3],
                                         rhs=rhs_ah, start=True, stop=True)

                    # gates: z|r sigmoid per chunk; u accumulated per batch elt
                    if half == 0:
                        zrb = elw.tile([128, 2 * N], bf16, tag="zr")
                        ub = elw.tile([128, N], bf16, tag="u")
                    zr = zrb[:, half * 2 * CHUNK:(half + 1) * 2 * CHUNK]
                    nc.scalar.activation(zr, pzr, AF.Sigmoid)
                    z = zr[:, 0:CHUNK]
                    r = zr[:, CHUNK:2 * CHUNK]
                    tq = elw.tile([128, CHUNK], bf16, tag="tq")
                    if first:
                        # tq = bhn * r      (nh == 0)
                        nc.vector.tensor_scalar_mul(tq, r, bhn_sb)
                    else:
                        # tq = (nh + bhn) * r
                        nc.vector.scalar_tensor_tensor(
                            tq, in0=pnh, scalar=bhn_sb, in1=r,
                            op0=ALU.add, op1=ALU.mult)
                    nc.vector.tensor_add(ub[:, is_], tq, pnx)

                    if half == 1:
                        # state update for the whole batch element at once;
                        # z lives at [half*2C : half*2C + C] so view as
                        # [p, half, C] APs to match shapes
                        bs = slice(b * N, (b + 1) * N)
                        nb_ = elw.tile([128, N], bf16, tag="n")
                        eb = elw.tile([128, N], bf16, tag="e")
                        z2 = zrb.rearrange("p (h two c) -> p two h c",
                                           two=2, c=CHUNK)[:, 0]
                        def v2(ap):
                            return ap.rearrange("p (h c) -> p h c", c=CHUNK)
                        nc.scalar.activation(nb_, ub, AF.Tanh)
                        if first:
                            # h_new = (1 - z) * n
                            nc.vector.tensor_mul(v2(eb), z2, v2(nb_))
                            nc.vector.tensor_sub(hrow_new[:, bs], nb_, eb)
                        else:
                            db = elw.tile([128, N], bf16, tag="d")
                            nc.gpsimd.tensor_sub(db, hrow[:, bs], nb_)
                            nc.vector.tensor_mul(v2(eb), z2, v2(db))
                            nc.vector.tensor_add(hrow_new[:, bs], nb_, eb)

                # transpose h_new back to Hmat layout: one xbar DMA per batch
                # element; out[p, jt, h] = h_new.T[jt*128+p, h]
                # (skipped on the last step: hmat is not consumed again)
                if l != L - 1:
                    for b in range(BLOC):
                        nc.sync.dma_start_transpose(
                            out=hmat_new[:, :, b * H:(b + 1) * H],
                            in_=hrow_new[:, b * N:(b + 1) * N])

                hmat = hmat_new
                hrow = hrow_new

            # ---- head ----
            for c in range(NCH):
                b, half = divmod(c, N // CHUNK)
                cs = slice(c * CHUNK, (c + 1) * CHUNK)
                is_ = slice(half * CHUNK, (half + 1) * CHUNK)
                ph = ps_nx.tile([HOR, CHUNK], f32, tag="pnx")
                nc.tensor.matmul(ph, lhsT=whead_sb, rhs=hrow[:, cs],
                                 start=True, stop=True)
                yc = yout.tile([HOR, CHUNK], f32, tag="yc")
                nc.scalar.activation(yc, ph, AF.Identity, bias=bhead_sb)
                nc.sync.dma_start(out=y[b][:, is_], in_=yc)

    nc.compile()
    return nc


def _prep_inputs(x, A, W_x, b_x, W_h, b_h, W_head, b_head):
    import ml_dtypes
    f = np.float32
    bf = ml_dtypes.bfloat16
    A_T = np.ascontiguousarray(A.T).astype(bf)
    bsum = (b_x + b_h).astype(f)
    # w_x with ones-row bias: [W_x0; W_x1; [bz | br | bxn]]
    bias_row = np.concatenate([bsum[0:2 * H], b_x[2 * H:G3].astype(f)])
    w_x_ext = np.concatenate([np.asarray(W_x, f),
                              bias_row.reshape(1, G3)], axis=0).astype(bf)
    common = {
        "a_t": A_T,
        "w_h": np.ascontiguousarray(W_h).astype(bf),
        "w_x": np.ascontiguousarray(w_x_ext),
        "bhn": np.ascontiguousarray(b_h[2 * H:G3], dtype=f).reshape(H, 1).copy(),
        "w_head": np.ascontiguousarray(W_head).astype(bf),
        "b_head": np.ascontiguousarray(b_head, dtype=f).reshape(HOR, 1).copy(),
    }
    in_maps = []
    for i in range(NCORES):
        xs = x[i * BLOC:(i + 1) * BLOC]          # [BLOC, L, N, F]
        xt_i = np.asarray(xs, f).transpose(1, 3, 0, 2).reshape(L, F, NB)
        xt_ext = np.concatenate(
            [xt_i, np.ones((L, 1, NB), f)], axis=1).astype(bf)
        in_maps.append({"xt": np.ascontiguousarray(xt_ext), **common})
    return in_maps


def kernel(x, A, W_x, b_x, W_h, b_h, W_head, b_head, _trace=False, _tmpdir=None):
    global _COMPILED
    from concourse.bass_utils import run_bass_kernel_spmd

    if _COMPILED is None:
        _COMPILED = _build_program()
    nc = _COMPILED

    in_maps = _prep_inputs(np.asarray(x), np.asarray(A), np.asarray(W_x),
                           np.asarray(b_x), np.asarray(W_h), np.asarray(b_h),
                           np.asarray(W_head), np.asarray(b_head))
    kw = {}
    if _trace:
        from concourse import bass_utils as _bu
        _bu.upload_artifacts = lambda tmpdir: tmpdir
        kw = dict(trace=True, tmpdir=_tmpdir)
    res = run_bass_kernel_spmd(nc, in_maps, list(range(NCORES)), **kw)

    out = np.empty((B, HOR, N), dtype=np.float32)
    for i in range(NCORES):
        out[i * BLOC:(i + 1) * BLOC] = res.results[i]["y"]
    if _trace:
        return out, res
    return out
